# revision 37
# baseline (speedup 1.0000x reference)
"""Trainium2 Bass kernel for the CapsuleNetwork routing problem (v2).

Problem (per reference):
  B, L, D, K = 1024, 200, 64, 4 ; E = K*D = 256
  hat[b,l,e] = sum_d seq[b,l,d] * W[l,e,d]          (einsum, PE)
  3 rounds of dynamic routing over interests K (softmax over K per (b,l)),
  cap = squash(w @ hat), cw += hat . cap
  output cap -> [B, K, D]

Sharding: pure data-parallel over batch across 8 NeuronCores (BS=128 rows
each); weights replicated. Host-side layout prep gives the device clean
burst DMAs and parity-major (par, m) slot order, l = 2m + par.

v2 architecture (vs the v1 3x-einsum-recompute kernel):
  * All inputs bf16 (halves DMA bytes); DMAs issued from SP + ACT + GPSIMD
    queues concurrently (transfers overlap across issuing engines).
  * hat computed ONCE (bf16, SBUF-resident, 100KiB/partition) -- einsum and
    the PSUM->SBUF copies happen once instead of 3x.
  * The over-l capacc reduction runs on the otherwise-idle PE as
    identity-matmul PSUM accumulation (fp32, better precision than bf16
    trees), freeing the DVE.
  * Remaining elementwise work (routing mults + delta d-folds) is split
    DVE/GPSIMD by per-chunk schedule tables (GPSIMD ~1.5x DVE cost/elem in
    the cost model but otherwise idle).
  * cw update + softmax run bulk per iteration (few big ops), not per chunk.

Restructured routing algebra (validated in v1 vs reference to ~3e-7):
  cw layout [B, slot, K];  w = exp(cw) / sum_k exp(cw)
  capRaw[b,(d,k)] = sum_slot w[b,slot,k] hat[b,slot,(d,k)]
  n = |capRaw|^2 ; s = n/(1+n)/sqrt(n+1e-9)
  cw += s[b,k] * (hat . capRaw)   (squash scale folded into the cw update)
  final out[b,(k,d)] = s * capRaw
"""

import os
import sys

import numpy as np

for _p in ("/opt/trn_rl_repo", "/root/.axon_site/_ro/trn_rl_repo"):
    if os.path.isdir(_p) and _p not in sys.path:
        sys.path.insert(0, _p)

B, L, D, K = 1024, 200, 64, 4
E = K * D
NCORES = 8
BS = B // NCORES  # 128 batch rows per core
M = L // 2        # l = 2m + par ; slot = par*M + m

MCW = 10   # m's per wT DMA chunk (10 chunks)
PSB = 4    # m's per einsum PSUM tile (single parity)
NL = 16    # slots per routing chunk -> 13 chunks (12x16 + 8)

NCHUNK = (L + NL - 1) // NL


def _sched(env, default):
    s = os.environ.get(env, default)
    out = s.split(",")
    assert len(out) == NCHUNK, f"{env}: need {NCHUNK} entries, got {len(out)}"
    return out


# Engine schedules: D=DVE, G=GPSIMD, P=PE(identity-matmul fold), A=ACT.
A_MULT = _sched("KERNEL_A_MULT", "D,D,D,D,D,D,D,D,D,D,G,G,G")
A_FOLD = _sched("KERNEL_A_FOLD", "P,D,P,G,D,P,P,D,P,D,P,D,P")
# phase-B whole-chunk engine assignment (fused per-chunk pipeline)
B_ENG = _sched("KERNEL_B_ENG", "D,D,G,D,D,G,D,G,D,G,D,G,D")
# einsum PSUM->SBUF hat copy engine: copies with idx%COPY_MOD in RES -> GPS
COPY_MOD = int(os.environ.get("KERNEL_COPY_MOD", "2"))
COPY_GPS_RES = tuple(
    int(x) for x in os.environ.get("KERNEL_COPY_GPS_RES", "1").split(",") if x != ""
)


def _chunk_slots(c):
    s0 = NL * c
    return s0, min(NL, L - s0)


def _chunk_ready_ci(c):
    """First wT-chunk index ci after which einsum has produced all hat slots
    of routing chunk c (einsum emits both parities per ci block)."""
    s0, nl = _chunk_slots(c)
    need = 0
    for s in range(s0, s0 + nl):
        m = s % M
        need = max(need, m // MCW)
    return need


def build_nc():
    """Build the Bass program for one core (SPMD; all cores run same NEFF)."""
    import concourse.bass as bass
    import concourse.tile as tile
    from concourse import bacc, mybir

    f32 = mybir.dt.float32
    bf16 = mybir.dt.bfloat16
    AF = mybir.ActivationFunctionType
    OP = mybir.AluOpType

    nc = bacc.Bacc(trn_type="TRN2", target_bir_lowering=False, debug=False)
    seqT_d = nc.dram_tensor("seqT", [128, M, BS], bf16, kind="ExternalInput")
    wT_d = nc.dram_tensor("wT", [128, M, E], bf16, kind="ExternalInput")
    cw_d = nc.dram_tensor("cw", [BS, L, K], f32, kind="ExternalInput")
    ident_d = nc.dram_tensor("ident", [128, 128], bf16, kind="ExternalInput")
    out_d = nc.dram_tensor("out", [BS, E], f32, kind="ExternalOutput")

    ENG = None  # set inside context

    with tile.TileContext(nc) as tc:
        with (
            tc.tile_pool(name="consts", bufs=1) as consts,
            tc.tile_pool(name="wtp", bufs=2) as wtp,
            tc.tile_pool(name="scrd", bufs=int(os.environ.get("KERNEL_SCRD", "4"))) as scrd,
            tc.tile_pool(name="scrg", bufs=int(os.environ.get("KERNEL_SCRG", "3"))) as scrg,
            tc.tile_pool(name="pse", bufs=3, space="PSUM") as pse,
            tc.tile_pool(name="psc", bufs=2, space="PSUM") as pscp,
        ):
            seqT = consts.tile([128, M, BS], bf16, name="seqT_sb")
            ident = consts.tile([128, 128], bf16, name="ident_sb")
            hat = consts.tile([BS, L, E], bf16, name="hat_sb")
            cw = consts.tile([BS, L, K], f32, name="cw_sb")
            wB = consts.tile([BS, L, K], bf16, name="wB_sb")
            zsum = consts.tile([BS, L], f32, name="zsum")
            zinv = consts.tile([BS, L], f32, name="zinv")
            deltaB = consts.tile([BS, L, K], f32, name="deltaB")
            capB = consts.tile([BS, E], bf16, name="capB")
            capRaw = consts.tile([BS, D, K], f32, name="capRaw")
            capAccD = (
                consts.tile([BS, D, K], f32, name="capAccD") if "D" in A_FOLD else None
            )
            capAccG = (
                consts.tile([BS, D, K], f32, name="capAccG") if "G" in A_FOLD else None
            )
            capOut = consts.tile([BS, E], f32, name="capOut")
            smalls = consts.tile([BS, 8, K], f32, name="smalls")
            nvec = smalls[:, 0, :]
            lnt = smalls[:, 1, :]
            rt = smalls[:, 2, :]
            np1 = smalls[:, 3, :]
            den = smalls[:, 4, :]
            dinv = smalls[:, 5, :]
            svec = smalls[:, 6, :]
            epsB = consts.tile([BS, 1], f32, name="epsB")
            u2s = consts.tile([BS, D], f32, name="u2s")
            nc.vector.memset(epsB[:], 1e-9)

            ENG = {"D": nc.vector, "G": nc.gpsimd}

            # ---------------- DMAs (parallel issue queues) ----------------
            # An engine-issued DMA blocks that engine for the whole transfer,
            # but transfers on different issuing engines overlap fully. cw
            # goes first on ACT (the initial softmax needs it), seqT is split
            # GPS/ACT, wT streams on SP under the einsum.
            nc.scalar.dma_start(out=cw[:], in_=cw_d[:])
            H = M // 2
            nc.gpsimd.dma_start(out=seqT[:, 0:H, :], in_=seqT_d[:, 0:H, :])

            # ---------------- helpers ----------------
            def wb_bcast(s0, nl):
                return bass.AP(
                    tensor=wB.tensor,
                    offset=wB.offset + s0 * K,
                    ap=[wB.ap[0], [K, nl], [0, D], [1, K]],
                )

            def capb_bcast(nl):
                return bass.AP(
                    tensor=capB.tensor,
                    offset=capB.offset,
                    ap=[capB.ap[0], [0, nl], [1, E]],
                )

            def zinv_bcast():
                return bass.AP(
                    tensor=zinv.tensor,
                    offset=zinv.offset,
                    ap=[zinv.ap[0], [1, L], [0, K]],
                )

            def softmax_range(s0, s1, eg=None):
                # exp straight into bf16 wB, then normalize in place
                n = s1 - s0
                zb = bass.AP(
                    tensor=zinv.tensor,
                    offset=zinv.offset + s0,
                    ap=[zinv.ap[0], [1, n], [0, K]],
                )
                nc.scalar.activation(
                    out=wB[:, s0:s1, :], in_=cw[:, s0:s1, :], func=AF.Exp
                )
                nc.vector.tensor_reduce(
                    out=zsum[:, s0:s1],
                    in_=wB[:, s0:s1, :],
                    axis=mybir.AxisListType.X,
                    op=OP.add,
                )
                nc.vector.reciprocal(out=zinv[:, s0:s1], in_=zsum[:, s0:s1])
                (eg or nc.vector).tensor_tensor(
                    out=wB[:, s0:s1, :], in0=wB[:, s0:s1, :], in1=zb, op=OP.mult
                )

            def softmax_bulk():
                softmax_range(0, L)

            def dk_col(t, k, n=D):
                # [BS, n] view of column k of a [BS, (d,k)] tile/psum region
                return bass.AP(
                    tensor=t.tensor, offset=t.offset + k, ap=[t.ap[0], [K, n]]
                )

            def squash(src, capb_src, capb_eng):
                # n[b,k] = sum_d src[b,d,k]^2 ; s = n/(1+n)/sqrt(n+1e-9)
                # src may be the PSUM accumulator directly (skips the copy on
                # the critical inter-iteration chain); capB copy runs off-chain.
                capb_eng.tensor_copy(out=capB[:], in_=capb_src)
                for k in range(K):
                    nc.vector.scalar_tensor_tensor(
                        out=u2s[:],
                        in0=dk_col(src, k),
                        scalar=1.0,
                        in1=dk_col(src, k),
                        op0=OP.mult,
                        op1=OP.mult,
                        accum_out=nvec[:, k : k + 1],
                    )
                nc.scalar.activation(out=lnt, in_=nvec, func=AF.Ln, bias=epsB[:])
                nc.scalar.activation(out=rt, in_=lnt, func=AF.Exp, scale=0.5)
                nc.vector.tensor_scalar_add(out=np1, in0=nvec, scalar1=1.0)
                nc.vector.tensor_mul(out=den, in0=np1, in1=rt)
                nc.vector.reciprocal(out=dinv, in_=den)
                nc.vector.tensor_mul(out=svec, in0=nvec, in1=dinv)

            def fold_tree_l(eng, u, nl, acc):
                """Sum u[:, 0:nl, :] over slots (nl power of 2), add into acc."""
                width = nl
                while width > 1:
                    h = width // 2
                    eng.tensor_tensor(
                        out=u[:, 0:h, :],
                        in0=u[:, 0:h, :],
                        in1=u[:, h : 2 * h, :],
                        op=OP.add,
                    )
                    width = h
                eng.tensor_tensor(
                    out=acc[:], in0=acc[:], in1=u[:, 0, :], op=OP.add
                )

            def fold_tree_d(eng, u, nl, s0):
                """delta[:, slot, k] = sum_d u[:, slot, (d,k)] -> deltaB."""
                width = D
                while width > 2:
                    h = width // 2
                    eng.tensor_tensor(
                        out=u[:, 0:nl, 0 : h * K],
                        in0=u[:, 0:nl, 0 : h * K],
                        in1=u[:, 0:nl, h * K : 2 * h * K],
                        op=OP.add,
                    )
                    width = h
                eng.tensor_tensor(
                    out=deltaB[:, s0 : s0 + nl, :],
                    in0=u[:, 0:nl, 0:K],
                    in1=u[:, 0:nl, K : 2 * K],
                    op=OP.add,
                )

            def utile(eng_key, tag):
                pool = scrd if eng_key == "D" else scrg
                return pool.tile([BS, NL, E], bf16, name=f"u{eng_key}", tag=f"u{eng_key}")

            # initial softmax (needs only cw); the ACT-issued seqT-half DMA is
            # emitted after the exp so the exp isn't queued behind it
            softmax_bulk()
            nc.scalar.dma_start(out=seqT[:, H:M, :], in_=seqT_d[:, H:M, :])
            if capAccD is not None:
                nc.vector.memset(capAccD[:], 0.0)
            if capAccG is not None:
                nc.gpsimd.memset(capAccG[:], 0.0)

            # ---------------- Phase A: einsum + hat + capacc_0 ----------------
            # chunk emission order by readiness
            order = sorted(range(NCHUNK), key=lambda c: (_chunk_ready_ci(c), c))
            pe_chunks = [c for c in order if A_FOLD[c] == "P"]
            psc0 = pscp.tile([128, 512], f32, name="psc", tag="psc")

            copy_idx = 0
            emitted = 0

            def emit_capacc0(c):
                nonlocal copy_idx
                s0, nl = _chunk_slots(c)
                me = A_MULT[c]
                u = utile(me, "a")
                ENG[me].tensor_tensor(
                    out=u[:, 0:nl, :],
                    in0=hat[:, s0 : s0 + nl, :],
                    in1=wb_bcast(s0, nl),
                    op=OP.mult,
                )
                fm = A_FOLD[c]
                if fm == "P":
                    first = c == pe_chunks[0]
                    last = c == pe_chunks[-1]
                    for j in range(nl):
                        nc.tensor.matmul(
                            psc0[:, 0:E],
                            lhsT=ident[:],
                            rhs=u[:, j, :],
                            start=(first and j == 0),
                            stop=(last and j == nl - 1),
                            skip_group_check=True,
                        )
                elif fm == "D":
                    fold_tree_l(nc.vector, u, nl, capAccD)
                else:
                    fold_tree_l(nc.gpsimd, u, nl, capAccG)

            for ci in range(M // MCW):
                m0 = ci * MCW
                wtc = wtp.tile([128, MCW, E], bf16, name="wtc", tag="wtc")
                nc.sync.dma_start(out=wtc[:], in_=wT_d[:, m0 : m0 + MCW, :])
                if ci == 0:
                    # ident is first needed by the PE folds (~12us in); keep it
                    # behind wT chunk 0 on the SP queue so einsum starts early
                    nc.sync.dma_start(out=ident[:], in_=ident_d[:])
                for par in (0, 1):
                    p0 = 64 * par
                    for g0 in range(0, MCW, PSB):
                        nb = min(PSB, MCW - g0)
                        ps = pse.tile([128, PSB, E], f32, name="pse", tag="pse")
                        for j in range(nb):
                            mo = g0 + j
                            nc.tensor.matmul(
                                ps[:, j, :],
                                lhsT=seqT[p0 : p0 + 64, m0 + mo, :],
                                rhs=wtc[p0 : p0 + 64, mo, :],
                                start=(j % 2 == 0),
                                stop=(j % 2 == 1 or j == nb - 1),
                                skip_group_check=True,
                            )
                        dst = hat[:, par * M + m0 + g0 : par * M + m0 + g0 + nb, :]
                        if copy_idx % COPY_MOD in COPY_GPS_RES:
                            nc.gpsimd.tensor_copy(out=dst, in_=ps[:, 0:nb, :])
                        else:
                            nc.scalar.copy(out=dst, in_=ps[:, 0:nb, :])
                        copy_idx += 1
                # emit routing chunks that are now fully covered
                while emitted < NCHUNK and _chunk_ready_ci(order[emitted]) <= ci:
                    emit_capacc0(order[emitted])
                    emitted += 1

            # capRaw = psc0 + tree partials
            nc.gpsimd.tensor_copy(out=capRaw[:], in_=psc0[:, 0:E])
            if "D" in A_FOLD:
                nc.vector.tensor_tensor(
                    out=capRaw[:], in0=capRaw[:], in1=capAccD[:], op=OP.add
                )
            if "G" in A_FOLD:
                nc.vector.tensor_tensor(
                    out=capRaw[:], in0=capRaw[:], in1=capAccG[:], op=OP.add
                )
            squash(
                capRaw,
                bass.AP(
                    tensor=capRaw.tensor,
                    offset=capRaw.offset,
                    ap=[capRaw.ap[0], [1, E]],
                ),
                nc.vector,
            )

            # ---------------- Phase B: iterations 1, 2 ----------------
            # Fully fused per-chunk pipeline: delta -> chunk-local cw update +
            # softmax -> capacc, whole chunk on one engine (exp on ACT, folds
            # of capacc on PE), so the PE ident-matmul stream and the ACT exps
            # hide under the DVE/GPSIMD streams with no bulk barriers.
            def svec_bcast(nl):
                return bass.AP(
                    tensor=smalls.tensor,
                    offset=smalls.offset + 6 * K,
                    ap=[smalls.ap[0], [0, nl], [1, K]],
                )

            def fused_chunk(c, psc, first, last):
                s0, nl = _chunk_slots(c)
                me = B_ENG[c]
                EG = ENG[me]
                u = utile(me, "b")
                EG.tensor_tensor(
                    out=u[:, 0:nl, :],
                    in0=hat[:, s0 : s0 + nl, :],
                    in1=capb_bcast(nl),
                    op=OP.mult,
                )
                fold_tree_d(EG, u, nl, s0)
                # cw_c += svec * deltaB_c (product built in deltaB in place)
                EG.tensor_tensor(
                    out=deltaB[:, s0 : s0 + nl, :],
                    in0=deltaB[:, s0 : s0 + nl, :],
                    in1=svec_bcast(nl),
                    op=OP.mult,
                )
                EG.tensor_tensor(
                    out=cw[:, s0 : s0 + nl, :],
                    in0=cw[:, s0 : s0 + nl, :],
                    in1=deltaB[:, s0 : s0 + nl, :],
                    op=OP.add,
                )
                # chunk softmax. exp on ACT; for GPSIMD chunks the k-sum and
                # normalize stay on GPSIMD (adds + divide) so the chain never
                # waits in the saturated DVE queue.
                if me == "G":
                    nc.scalar.activation(
                        out=wB[:, s0 : s0 + nl, :],
                        in_=cw[:, s0 : s0 + nl, :],
                        func=AF.Exp,
                    )
                    zs = zsum[:, s0 : s0 + nl]

                    def k_col(k):
                        return bass.AP(
                            tensor=wB.tensor,
                            offset=wB.offset + s0 * K + k,
                            ap=[wB.ap[0], [K, nl]],
                        )

                    EG.tensor_tensor(out=zs, in0=k_col(0), in1=k_col(1), op=OP.add)
                    EG.tensor_tensor(out=zs, in0=zs, in1=k_col(2), op=OP.add)
                    EG.tensor_tensor(out=zs, in0=zs, in1=k_col(3), op=OP.add)
                    zb = bass.AP(
                        tensor=zsum.tensor,
                        offset=zsum.offset + s0,
                        ap=[zsum.ap[0], [1, nl], [0, K]],
                    )
                    EG.tensor_tensor(
                        out=wB[:, s0 : s0 + nl, :],
                        in0=wB[:, s0 : s0 + nl, :],
                        in1=zb,
                        op=OP.divide,
                    )
                else:
                    softmax_range(s0, s0 + nl, eg=EG)
                # capacc
                u3 = utile(me, "b")
                EG.tensor_tensor(
                    out=u3[:, 0:nl, :],
                    in0=hat[:, s0 : s0 + nl, :],
                    in1=wb_bcast(s0, nl),
                    op=OP.mult,
                )
                for j in range(nl):
                    nc.tensor.matmul(
                        psc[:, 0:E],
                        lhsT=ident[:],
                        rhs=u3[:, j, :],
                        start=(first and j == 0),
                        stop=(last and j == nl - 1),
                        skip_group_check=True,
                    )

            # GPSIMD chunks emitted first: GPSIMD's chain is ~40% slower per
            # chunk, so giving it a queue head start keeps it off the
            # iteration-end critical path (the last emitted chunk is DVE).
            border = [c for c in range(NCHUNK) if B_ENG[c] == "G"] + [
                c for c in range(NCHUNK) if B_ENG[c] != "G"
            ]
            for it in (1, 2):
                psc = pscp.tile([128, 512], f32, name="psc", tag="psc")
                for i, c in enumerate(border):
                    fused_chunk(c, psc, first=(i == 0), last=(i == NCHUNK - 1))
                squash(psc, psc[:, 0:E], nc.gpsimd)
                if it == 2:
                    # final: out[b, (k,d)] = s[b,k] * psc[b, (d,k)]
                    for k in range(K):
                        nc.vector.tensor_scalar_mul(
                            out=capOut[:, k * D : (k + 1) * D],
                            in0=dk_col(psc, k),
                            scalar1=svec[:, k : k + 1],
                        )
            nc.sync.dma_start(out=out_d[:], in_=capOut[:])

    nc.finalize()
    return nc


_NC_CACHE = None


def _get_nc():
    global _NC_CACHE
    if _NC_CACHE is None:
        _NC_CACHE = build_nc()
    return _NC_CACHE


def prep_inputs(seq_out, weights, capsule_weight):
    """Host-side layout prep -> list of per-core input maps."""
    import ml_dtypes

    bf16 = ml_dtypes.bfloat16
    seq = np.ascontiguousarray(np.asarray(seq_out, dtype=np.float32))
    W = np.ascontiguousarray(np.asarray(weights, dtype=np.float32))[0]  # [L,E,D]
    cwf = np.ascontiguousarray(np.asarray(capsule_weight, dtype=np.float32))

    # seqT[p=(64*par+d'), m, b] = seq[b, 2m+par, d']
    seqT = np.ascontiguousarray(
        seq.reshape(B, M, 2, D).transpose(2, 3, 1, 0).reshape(128, M, B).astype(bf16)
    )
    # wT[p, m, (d*K+k)] = W[2m+par, k*D+d, d']
    wTf = W.reshape(M, 2, K, D, D).transpose(1, 4, 0, 3, 2)  # [par, d', m, d, k]
    wT = np.ascontiguousarray(wTf.reshape(128, M, E).astype(bf16))
    # cwA[b, slot=(par*M+m), k] = cw[b, k, 2m+par]
    cwA = np.ascontiguousarray(
        cwf.reshape(B, K, M, 2).transpose(0, 3, 2, 1).reshape(B, L, K)
    )
    ident = np.eye(128, dtype=bf16)

    in_maps = []
    for c in range(NCORES):
        in_maps.append(
            {
                "seqT": np.ascontiguousarray(seqT[:, :, c * BS : (c + 1) * BS]),
                "wT": wT,
                "cw": np.ascontiguousarray(cwA[c * BS : (c + 1) * BS]),
                "ident": ident,
            }
        )
    return in_maps


def gather_out(results):
    """Per-core 'out' [BS, E=(k*D+d)] -> full [B, K, D]."""
    return np.concatenate(
        [np.asarray(r["out"]).reshape(BS, K, D) for r in results], axis=0
    ).astype(np.float32)


def kernel(seq_out, mask, weights, capsule_weight):
    from concourse.bass_utils import run_bass_kernel_spmd

    nc = _get_nc()
    in_maps = prep_inputs(seq_out, weights, capsule_weight)
    res = run_bass_kernel_spmd(nc, in_maps, core_ids=list(range(NCORES)))
    return gather_out(res.results)


if __name__ == "__main__":
    rng = np.random.default_rng(0)
    seq_out = rng.standard_normal((B, L, D), dtype=np.float32)
    mask = np.ones((B, L), dtype=np.float32)
    weights = (0.02 * rng.standard_normal((1, L, E, D))).astype(np.float32)
    capsule_weight = rng.standard_normal((B, K, L)).astype(np.float32)
    out = kernel(seq_out, mask, weights, capsule_weight)
    print("out", out.shape, out.dtype, float(np.abs(out).max()))


# revision 39
# speedup vs baseline: 1.2682x; 1.2682x over previous
"""Trainium2 Bass kernel for the CapsuleNetwork routing problem (v2).

Problem (per reference):
  B, L, D, K = 1024, 200, 64, 4 ; E = K*D = 256
  hat[b,l,e] = sum_d seq[b,l,d] * W[l,e,d]          (einsum, PE)
  3 rounds of dynamic routing over interests K (softmax over K per (b,l)),
  cap = squash(w @ hat), cw += hat . cap
  output cap -> [B, K, D]

Sharding: pure data-parallel over batch across 8 NeuronCores (BS=128 rows
each); weights replicated. Host-side layout prep gives the device clean
burst DMAs and parity-major (par, m) slot order, l = 2m + par.

v2 architecture (vs the v1 3x-einsum-recompute kernel):
  * All inputs bf16 (halves DMA bytes); DMAs issued from SP + ACT + GPSIMD
    queues concurrently (transfers overlap across issuing engines).
  * hat computed ONCE (bf16, SBUF-resident, 100KiB/partition) -- einsum and
    the PSUM->SBUF copies happen once instead of 3x.
  * The over-l capacc reduction runs on the otherwise-idle PE as
    identity-matmul PSUM accumulation (fp32, better precision than bf16
    trees), freeing the DVE.
  * Remaining elementwise work (routing mults + delta d-folds) is split
    DVE/GPSIMD by per-chunk schedule tables (GPSIMD ~1.5x DVE cost/elem in
    the cost model but otherwise idle).
  * cw update + softmax run bulk per iteration (few big ops), not per chunk.

Restructured routing algebra (validated in v1 vs reference to ~3e-7):
  cw layout [B, slot, K];  w = exp(cw) / sum_k exp(cw)
  capRaw[b,(d,k)] = sum_slot w[b,slot,k] hat[b,slot,(d,k)]
  n = |capRaw|^2 ; s = n/(1+n)/sqrt(n+1e-9)
  cw += s[b,k] * (hat . capRaw)   (squash scale folded into the cw update)
  final out[b,(k,d)] = s * capRaw
"""

import os
import sys

import numpy as np

for _p in ("/opt/trn_rl_repo", "/root/.axon_site/_ro/trn_rl_repo"):
    if os.path.isdir(_p) and _p not in sys.path:
        sys.path.insert(0, _p)

B, L, D, K = 1024, 200, 64, 4
E = K * D
NCORES = 8
BS = B // NCORES  # 128 batch rows per core
M = L // 2        # l = 2m + par ; slot = par*M + m

MCW = 10   # m's per wT DMA chunk (10 chunks)
PSB = 4    # m's per einsum PSUM tile (single parity)
NL = 16    # slots per routing chunk -> 13 chunks (12x16 + 8)

NCHUNK = (L + NL - 1) // NL


def _sched(env, default):
    s = os.environ.get(env, default)
    out = s.split(",")
    assert len(out) == NCHUNK, f"{env}: need {NCHUNK} entries, got {len(out)}"
    return out


# Engine schedules: D=DVE, G=GPSIMD, P=PE(identity-matmul fold), A=ACT.
A_MULT = _sched("KERNEL_A_MULT", "D,D,D,D,D,D,D,D,D,D,G,G,G")
A_FOLD = _sched("KERNEL_A_FOLD", "P,D,P,G,D,P,P,D,P,D,P,D,P")
# phase-B whole-chunk engine assignment (fused per-chunk pipeline)
B_ENG = _sched("KERNEL_B_ENG", "D,G,D,G,D,G,D,G,D,G,D,D,D")
# einsum PSUM->SBUF hat copy engine: copies with idx%COPY_MOD in RES -> GPS
COPY_MOD = int(os.environ.get("KERNEL_COPY_MOD", "2"))
COPY_GPS_RES = tuple(
    int(x) for x in os.environ.get("KERNEL_COPY_GPS_RES", "1").split(",") if x != ""
)


def _chunk_slots(c):
    s0 = NL * c
    return s0, min(NL, L - s0)


def _chunk_ready_ci(c):
    """First wT-chunk index ci after which einsum has produced all hat slots
    of routing chunk c (einsum emits both parities per ci block)."""
    s0, nl = _chunk_slots(c)
    need = 0
    for s in range(s0, s0 + nl):
        m = s % M
        need = max(need, m // MCW)
    return need


def build_nc():
    """Build the Bass program for one core (SPMD; all cores run same NEFF)."""
    import concourse.bass as bass
    import concourse.tile as tile
    from concourse import bacc, mybir

    f32 = mybir.dt.float32
    bf16 = mybir.dt.bfloat16
    AF = mybir.ActivationFunctionType
    OP = mybir.AluOpType

    nc = bacc.Bacc(trn_type="TRN2", target_bir_lowering=False, debug=False)
    seqT_d = nc.dram_tensor("seqT", [128, M, BS], bf16, kind="ExternalInput")
    wT_d = nc.dram_tensor("wT", [128, M, E], bf16, kind="ExternalInput")
    cw_d = nc.dram_tensor("cw", [BS, L, K], f32, kind="ExternalInput")
    ident_d = nc.dram_tensor("ident", [128, 128], bf16, kind="ExternalInput")
    out_d = nc.dram_tensor("out", [BS, E], f32, kind="ExternalOutput")

    ENG = None  # set inside context

    with tile.TileContext(nc) as tc:
        with (
            tc.tile_pool(name="consts", bufs=1) as consts,
            tc.tile_pool(name="wtp", bufs=2) as wtp,
            tc.tile_pool(name="scrd", bufs=int(os.environ.get("KERNEL_SCRD", "4"))) as scrd,
            tc.tile_pool(name="scrg", bufs=int(os.environ.get("KERNEL_SCRG", "3"))) as scrg,
            tc.tile_pool(name="pse", bufs=3, space="PSUM") as pse,
            tc.tile_pool(name="psc", bufs=2, space="PSUM") as pscp,
        ):
            seqT = consts.tile([128, M, BS], bf16, name="seqT_sb")
            ident = consts.tile([128, 128], bf16, name="ident_sb")
            hat = consts.tile([BS, L, E], bf16, name="hat_sb")
            cw = consts.tile([BS, L, K], f32, name="cw_sb")
            wB = consts.tile([BS, L, K], bf16, name="wB_sb")
            zsum = consts.tile([BS, L], f32, name="zsum")
            zinv = consts.tile([BS, L], f32, name="zinv")
            deltaB = consts.tile([BS, L, K], f32, name="deltaB")
            capB = consts.tile([BS, E], bf16, name="capB")
            capRaw = consts.tile([BS, D, K], f32, name="capRaw")
            capAccD = (
                consts.tile([BS, D, K], f32, name="capAccD") if "D" in A_FOLD else None
            )
            capAccG = (
                consts.tile([BS, D, K], f32, name="capAccG") if "G" in A_FOLD else None
            )
            capOut = consts.tile([BS, E], f32, name="capOut")
            smalls = consts.tile([BS, 8, K], f32, name="smalls")
            nvec = smalls[:, 0, :]
            lnt = smalls[:, 1, :]
            rt = smalls[:, 2, :]
            np1 = smalls[:, 3, :]
            den = smalls[:, 4, :]
            dinv = smalls[:, 5, :]
            svec = smalls[:, 6, :]
            epsB = consts.tile([BS, 1], f32, name="epsB")
            u2s = consts.tile([BS, D], f32, name="u2s")
            nc.vector.memset(epsB[:], 1e-9)

            ENG = {"D": nc.vector, "G": nc.gpsimd}

            # ---------------- DMAs (parallel issue queues) ----------------
            # An engine-issued DMA blocks that engine for the whole transfer,
            # but transfers on different issuing engines overlap fully. cw
            # goes first on ACT (the initial softmax needs it), seqT is split
            # GPS/ACT, wT streams on SP under the einsum.
            nc.scalar.dma_start(out=cw[:], in_=cw_d[:])
            H = M // 2
            nc.gpsimd.dma_start(out=seqT[:, 0:H, :], in_=seqT_d[:, 0:H, :])

            # ---------------- helpers ----------------
            def wb_bcast(s0, nl):
                return bass.AP(
                    tensor=wB.tensor,
                    offset=wB.offset + s0 * K,
                    ap=[wB.ap[0], [K, nl], [0, D], [1, K]],
                )

            def capb_bcast(nl):
                return bass.AP(
                    tensor=capB.tensor,
                    offset=capB.offset,
                    ap=[capB.ap[0], [0, nl], [1, E]],
                )

            def zinv_bcast():
                return bass.AP(
                    tensor=zinv.tensor,
                    offset=zinv.offset,
                    ap=[zinv.ap[0], [1, L], [0, K]],
                )

            def softmax_range(s0, s1, eg=None):
                # exp straight into bf16 wB, then normalize in place
                n = s1 - s0
                zb = bass.AP(
                    tensor=zinv.tensor,
                    offset=zinv.offset + s0,
                    ap=[zinv.ap[0], [1, n], [0, K]],
                )
                nc.scalar.activation(
                    out=wB[:, s0:s1, :], in_=cw[:, s0:s1, :], func=AF.Exp
                )
                nc.vector.tensor_reduce(
                    out=zsum[:, s0:s1],
                    in_=wB[:, s0:s1, :],
                    axis=mybir.AxisListType.X,
                    op=OP.add,
                )
                nc.vector.reciprocal(out=zinv[:, s0:s1], in_=zsum[:, s0:s1])
                (eg or nc.vector).tensor_tensor(
                    out=wB[:, s0:s1, :], in0=wB[:, s0:s1, :], in1=zb, op=OP.mult
                )

            def softmax_bulk():
                softmax_range(0, L)

            def dk_col(t, k, n=D):
                # [BS, n] view of column k of a [BS, (d,k)] tile/psum region
                return bass.AP(
                    tensor=t.tensor, offset=t.offset + k, ap=[t.ap[0], [K, n]]
                )

            def squash(src, capb_src, capb_eng):
                # n[b,k] = sum_d src[b,d,k]^2 ; s = n/(1+n)/sqrt(n+1e-9)
                # src may be the PSUM accumulator directly (skips the copy on
                # the critical inter-iteration chain); capB copy runs off-chain.
                capb_eng.tensor_copy(out=capB[:], in_=capb_src)
                for k in range(K):
                    nc.vector.scalar_tensor_tensor(
                        out=u2s[:],
                        in0=dk_col(src, k),
                        scalar=1.0,
                        in1=dk_col(src, k),
                        op0=OP.mult,
                        op1=OP.mult,
                        accum_out=nvec[:, k : k + 1],
                    )
                nc.scalar.activation(out=lnt, in_=nvec, func=AF.Ln, bias=epsB[:])
                nc.scalar.activation(out=rt, in_=lnt, func=AF.Exp, scale=0.5)
                nc.vector.tensor_scalar_add(out=np1, in0=nvec, scalar1=1.0)
                nc.vector.tensor_mul(out=den, in0=np1, in1=rt)
                nc.vector.reciprocal(out=dinv, in_=den)
                nc.vector.tensor_mul(out=svec, in0=nvec, in1=dinv)

            def fold_tree_l(eng, u, nl, acc):
                """Sum u[:, 0:nl, :] over slots (nl power of 2), add into acc."""
                width = nl
                while width > 1:
                    h = width // 2
                    eng.tensor_tensor(
                        out=u[:, 0:h, :],
                        in0=u[:, 0:h, :],
                        in1=u[:, h : 2 * h, :],
                        op=OP.add,
                    )
                    width = h
                eng.tensor_tensor(
                    out=acc[:], in0=acc[:], in1=u[:, 0, :], op=OP.add
                )

            def fold_tree_d(eng, u, nl, s0):
                """delta[:, slot, k] = sum_d u[:, slot, (d,k)] -> deltaB."""
                width = D
                while width > 2:
                    h = width // 2
                    eng.tensor_tensor(
                        out=u[:, 0:nl, 0 : h * K],
                        in0=u[:, 0:nl, 0 : h * K],
                        in1=u[:, 0:nl, h * K : 2 * h * K],
                        op=OP.add,
                    )
                    width = h
                eng.tensor_tensor(
                    out=deltaB[:, s0 : s0 + nl, :],
                    in0=u[:, 0:nl, 0:K],
                    in1=u[:, 0:nl, K : 2 * K],
                    op=OP.add,
                )

            def utile(eng_key, tag):
                pool = scrd if eng_key == "D" else scrg
                return pool.tile([BS, NL, E], bf16, name=f"u{eng_key}", tag=f"u{eng_key}")

            # initial softmax (needs only cw); the ACT-issued seqT-half DMA is
            # emitted after the exp so the exp isn't queued behind it
            softmax_bulk()
            nc.scalar.dma_start(out=seqT[:, H:M, :], in_=seqT_d[:, H:M, :])
            if capAccD is not None:
                nc.vector.memset(capAccD[:], 0.0)
            if capAccG is not None:
                nc.gpsimd.memset(capAccG[:], 0.0)

            # ---------------- Phase A: einsum + hat + capacc_0 ----------------
            # chunk emission order by readiness
            order = sorted(range(NCHUNK), key=lambda c: (_chunk_ready_ci(c), c))
            pe_chunks = [c for c in order if A_FOLD[c] == "P"]
            psc0 = pscp.tile([128, 512], f32, name="psc", tag="psc")

            copy_idx = 0
            emitted = 0

            def emit_capacc0(c):
                nonlocal copy_idx
                s0, nl = _chunk_slots(c)
                me = A_MULT[c]
                u = utile(me, "a")
                ENG[me].tensor_tensor(
                    out=u[:, 0:nl, :],
                    in0=hat[:, s0 : s0 + nl, :],
                    in1=wb_bcast(s0, nl),
                    op=OP.mult,
                )
                fm = A_FOLD[c]
                if fm == "P":
                    first = c == pe_chunks[0]
                    last = c == pe_chunks[-1]
                    for j in range(nl):
                        nc.tensor.matmul(
                            psc0[:, 0:E],
                            lhsT=ident[:],
                            rhs=u[:, j, :],
                            start=(first and j == 0),
                            stop=(last and j == nl - 1),
                            skip_group_check=True,
                        )
                elif fm == "D":
                    fold_tree_l(nc.vector, u, nl, capAccD)
                else:
                    fold_tree_l(nc.gpsimd, u, nl, capAccG)

            for ci in range(M // MCW):
                m0 = ci * MCW
                wtc = wtp.tile([128, MCW, E], bf16, name="wtc", tag="wtc")
                nc.sync.dma_start(out=wtc[:], in_=wT_d[:, m0 : m0 + MCW, :])
                if ci == 0:
                    # ident is first needed by the PE folds (~12us in); keep it
                    # behind wT chunk 0 on the SP queue so einsum starts early
                    nc.sync.dma_start(out=ident[:], in_=ident_d[:])
                for par in (0, 1):
                    p0 = 64 * par
                    for g0 in range(0, MCW, PSB):
                        nb = min(PSB, MCW - g0)
                        ps = pse.tile([128, PSB, E], f32, name="pse", tag="pse")
                        for j in range(nb):
                            mo = g0 + j
                            nc.tensor.matmul(
                                ps[:, j, :],
                                lhsT=seqT[p0 : p0 + 64, m0 + mo, :],
                                rhs=wtc[p0 : p0 + 64, mo, :],
                                start=(j % 2 == 0),
                                stop=(j % 2 == 1 or j == nb - 1),
                                skip_group_check=True,
                            )
                        dst = hat[:, par * M + m0 + g0 : par * M + m0 + g0 + nb, :]
                        if copy_idx % COPY_MOD in COPY_GPS_RES:
                            nc.gpsimd.tensor_copy(out=dst, in_=ps[:, 0:nb, :])
                        else:
                            nc.scalar.copy(out=dst, in_=ps[:, 0:nb, :])
                        copy_idx += 1
                # emit routing chunks that are now fully covered
                while emitted < NCHUNK and _chunk_ready_ci(order[emitted]) <= ci:
                    emit_capacc0(order[emitted])
                    emitted += 1

            # capRaw = psc0 + tree partials
            nc.gpsimd.tensor_copy(out=capRaw[:], in_=psc0[:, 0:E])
            if "D" in A_FOLD:
                nc.vector.tensor_tensor(
                    out=capRaw[:], in0=capRaw[:], in1=capAccD[:], op=OP.add
                )
            if "G" in A_FOLD:
                nc.vector.tensor_tensor(
                    out=capRaw[:], in0=capRaw[:], in1=capAccG[:], op=OP.add
                )
            squash(
                capRaw,
                bass.AP(
                    tensor=capRaw.tensor,
                    offset=capRaw.offset,
                    ap=[capRaw.ap[0], [1, E]],
                ),
                nc.vector,
            )

            # ---------------- Phase B: iterations 1, 2 ----------------
            # Fully fused per-chunk pipeline: delta -> chunk-local cw update +
            # softmax -> capacc, whole chunk on one engine (exp on ACT, folds
            # of capacc on PE), so the PE ident-matmul stream and the ACT exps
            # hide under the DVE/GPSIMD streams with no bulk barriers.
            def svec_bcast(nl):
                return bass.AP(
                    tensor=smalls.tensor,
                    offset=smalls.offset + 6 * K,
                    ap=[smalls.ap[0], [0, nl], [1, K]],
                )

            def fused_chunk(c, psc, first, last):
                s0, nl = _chunk_slots(c)
                me = B_ENG[c]
                EG = ENG[me]
                u = utile(me, "b")
                EG.tensor_tensor(
                    out=u[:, 0:nl, :],
                    in0=hat[:, s0 : s0 + nl, :],
                    in1=capb_bcast(nl),
                    op=OP.mult,
                )
                fold_tree_d(EG, u, nl, s0)
                # cw_c += svec * deltaB_c (product built in deltaB in place)
                EG.tensor_tensor(
                    out=deltaB[:, s0 : s0 + nl, :],
                    in0=deltaB[:, s0 : s0 + nl, :],
                    in1=svec_bcast(nl),
                    op=OP.mult,
                )
                EG.tensor_tensor(
                    out=cw[:, s0 : s0 + nl, :],
                    in0=cw[:, s0 : s0 + nl, :],
                    in1=deltaB[:, s0 : s0 + nl, :],
                    op=OP.add,
                )
                # chunk softmax. exp on ACT; for GPSIMD chunks the k-sum and
                # normalize stay on GPSIMD (adds + divide) so the chain never
                # waits in the saturated DVE queue.
                if me == "G":
                    nc.scalar.activation(
                        out=wB[:, s0 : s0 + nl, :],
                        in_=cw[:, s0 : s0 + nl, :],
                        func=AF.Exp,
                    )
                    zs = zsum[:, s0 : s0 + nl]

                    def k_col(k):
                        return bass.AP(
                            tensor=wB.tensor,
                            offset=wB.offset + s0 * K + k,
                            ap=[wB.ap[0], [K, nl]],
                        )

                    EG.tensor_tensor(out=zs, in0=k_col(0), in1=k_col(1), op=OP.add)
                    EG.tensor_tensor(out=zs, in0=zs, in1=k_col(2), op=OP.add)
                    EG.tensor_tensor(out=zs, in0=zs, in1=k_col(3), op=OP.add)
                    zb = bass.AP(
                        tensor=zsum.tensor,
                        offset=zsum.offset + s0,
                        ap=[zsum.ap[0], [1, nl], [0, K]],
                    )
                    EG.tensor_tensor(
                        out=wB[:, s0 : s0 + nl, :],
                        in0=wB[:, s0 : s0 + nl, :],
                        in1=zb,
                        op=OP.divide,
                    )
                else:
                    softmax_range(s0, s0 + nl, eg=EG)
                # capacc
                u3 = utile(me, "b")
                EG.tensor_tensor(
                    out=u3[:, 0:nl, :],
                    in0=hat[:, s0 : s0 + nl, :],
                    in1=wb_bcast(s0, nl),
                    op=OP.mult,
                )
                for j in range(nl):
                    nc.tensor.matmul(
                        psc[:, 0:E],
                        lhsT=ident[:],
                        rhs=u3[:, j, :],
                        start=(first and j == 0),
                        stop=(last and j == nl - 1),
                        skip_group_check=True,
                    )

            for it in (1, 2):
                psc = pscp.tile([128, 512], f32, name="psc", tag="psc")
                for c in range(NCHUNK):
                    fused_chunk(c, psc, first=(c == 0), last=(c == NCHUNK - 1))
                squash(psc, psc[:, 0:E], nc.gpsimd)
                if it == 2:
                    # final: out[b, (k,d)] = s[b,k] * psc[b, (d,k)]
                    for k in range(K):
                        nc.vector.tensor_scalar_mul(
                            out=capOut[:, k * D : (k + 1) * D],
                            in0=dk_col(psc, k),
                            scalar1=svec[:, k : k + 1],
                        )
            nc.sync.dma_start(out=out_d[:], in_=capOut[:])

    nc.finalize()
    return nc


_NC_CACHE = None


def _get_nc():
    global _NC_CACHE
    if _NC_CACHE is None:
        _NC_CACHE = build_nc()
    return _NC_CACHE


def prep_inputs(seq_out, weights, capsule_weight):
    """Host-side layout prep -> list of per-core input maps."""
    import ml_dtypes

    bf16 = ml_dtypes.bfloat16
    seq = np.ascontiguousarray(np.asarray(seq_out, dtype=np.float32))
    W = np.ascontiguousarray(np.asarray(weights, dtype=np.float32))[0]  # [L,E,D]
    cwf = np.ascontiguousarray(np.asarray(capsule_weight, dtype=np.float32))

    # seqT[p=(64*par+d'), m, b] = seq[b, 2m+par, d']
    seqT = np.ascontiguousarray(
        seq.reshape(B, M, 2, D).transpose(2, 3, 1, 0).reshape(128, M, B).astype(bf16)
    )
    # wT[p, m, (d*K+k)] = W[2m+par, k*D+d, d']
    wTf = W.reshape(M, 2, K, D, D).transpose(1, 4, 0, 3, 2)  # [par, d', m, d, k]
    wT = np.ascontiguousarray(wTf.reshape(128, M, E).astype(bf16))
    # cwA[b, slot=(par*M+m), k] = cw[b, k, 2m+par]
    cwA = np.ascontiguousarray(
        cwf.reshape(B, K, M, 2).transpose(0, 3, 2, 1).reshape(B, L, K)
    )
    ident = np.eye(128, dtype=bf16)

    in_maps = []
    for c in range(NCORES):
        in_maps.append(
            {
                "seqT": np.ascontiguousarray(seqT[:, :, c * BS : (c + 1) * BS]),
                "wT": wT,
                "cw": np.ascontiguousarray(cwA[c * BS : (c + 1) * BS]),
                "ident": ident,
            }
        )
    return in_maps


def gather_out(results):
    """Per-core 'out' [BS, E=(k*D+d)] -> full [B, K, D]."""
    return np.concatenate(
        [np.asarray(r["out"]).reshape(BS, K, D) for r in results], axis=0
    ).astype(np.float32)


def kernel(seq_out, mask, weights, capsule_weight):
    from concourse.bass_utils import run_bass_kernel_spmd

    nc = _get_nc()
    in_maps = prep_inputs(seq_out, weights, capsule_weight)
    res = run_bass_kernel_spmd(nc, in_maps, core_ids=list(range(NCORES)))
    return gather_out(res.results)


if __name__ == "__main__":
    rng = np.random.default_rng(0)
    seq_out = rng.standard_normal((B, L, D), dtype=np.float32)
    mask = np.ones((B, L), dtype=np.float32)
    weights = (0.02 * rng.standard_normal((1, L, E, D))).astype(np.float32)
    capsule_weight = rng.standard_normal((B, K, L)).astype(np.float32)
    out = kernel(seq_out, mask, weights, capsule_weight)
    print("out", out.shape, out.dtype, float(np.abs(out).max()))


# revision 40
# speedup vs baseline: 1.2774x; 1.0072x over previous
"""Trainium2 Bass kernel for the CapsuleNetwork routing problem (v2).

Problem (per reference):
  B, L, D, K = 1024, 200, 64, 4 ; E = K*D = 256
  hat[b,l,e] = sum_d seq[b,l,d] * W[l,e,d]          (einsum, PE)
  3 rounds of dynamic routing over interests K (softmax over K per (b,l)),
  cap = squash(w @ hat), cw += hat . cap
  output cap -> [B, K, D]

Sharding: pure data-parallel over batch across 8 NeuronCores (BS=128 rows
each); weights replicated. Host-side layout prep gives the device clean
burst DMAs and parity-major (par, m) slot order, l = 2m + par.

v2 architecture (vs the v1 3x-einsum-recompute kernel):
  * All inputs bf16 (halves DMA bytes); DMAs issued from SP + ACT + GPSIMD
    queues concurrently (transfers overlap across issuing engines).
  * hat computed ONCE (bf16, SBUF-resident, 100KiB/partition) -- einsum and
    the PSUM->SBUF copies happen once instead of 3x.
  * The over-l capacc reduction runs on the otherwise-idle PE as
    identity-matmul PSUM accumulation (fp32, better precision than bf16
    trees), freeing the DVE.
  * Remaining elementwise work (routing mults + delta d-folds) is split
    DVE/GPSIMD by per-chunk schedule tables (GPSIMD ~1.5x DVE cost/elem in
    the cost model but otherwise idle).
  * cw update + softmax run bulk per iteration (few big ops), not per chunk.

Restructured routing algebra (validated in v1 vs reference to ~3e-7):
  cw layout [B, slot, K];  w = exp(cw) / sum_k exp(cw)
  capRaw[b,(d,k)] = sum_slot w[b,slot,k] hat[b,slot,(d,k)]
  n = |capRaw|^2 ; s = n/(1+n)/sqrt(n+1e-9)
  cw += s[b,k] * (hat . capRaw)   (squash scale folded into the cw update)
  final out[b,(k,d)] = s * capRaw
"""

import os
import sys

import numpy as np

for _p in ("/opt/trn_rl_repo", "/root/.axon_site/_ro/trn_rl_repo"):
    if os.path.isdir(_p) and _p not in sys.path:
        sys.path.insert(0, _p)

B, L, D, K = 1024, 200, 64, 4
E = K * D
NCORES = 8
BS = B // NCORES  # 128 batch rows per core
M = L // 2        # l = 2m + par ; slot = par*M + m

MCW = 10   # m's per wT DMA chunk (10 chunks)
PSB = 4    # m's per einsum PSUM tile (single parity)
NL = 16    # slots per routing chunk -> 13 chunks (12x16 + 8)

NCHUNK = (L + NL - 1) // NL


def _sched(env, default):
    s = os.environ.get(env, default)
    out = s.split(",")
    assert len(out) == NCHUNK, f"{env}: need {NCHUNK} entries, got {len(out)}"
    return out


# Engine schedules: D=DVE, G=GPSIMD, P=PE(identity-matmul fold), A=ACT.
A_MULT = _sched("KERNEL_A_MULT", "D,D,D,D,D,D,D,D,D,D,D,G,G")
A_FOLD = _sched("KERNEL_A_FOLD", "P,D,P,G,D,P,P,D,P,D,P,D,P")
# phase-B whole-chunk engine assignment (fused per-chunk pipeline)
B_ENG = _sched("KERNEL_B_ENG", "D,G,D,G,D,G,D,G,D,G,D,D,D")
# einsum PSUM->SBUF hat copy engine: copies with idx%COPY_MOD in RES -> GPS
COPY_MOD = int(os.environ.get("KERNEL_COPY_MOD", "2"))
COPY_GPS_RES = tuple(
    int(x) for x in os.environ.get("KERNEL_COPY_GPS_RES", "1").split(",") if x != ""
)


def _chunk_slots(c):
    s0 = NL * c
    return s0, min(NL, L - s0)


def _chunk_ready_ci(c):
    """First wT-chunk index ci after which einsum has produced all hat slots
    of routing chunk c (einsum emits both parities per ci block)."""
    s0, nl = _chunk_slots(c)
    need = 0
    for s in range(s0, s0 + nl):
        m = s % M
        need = max(need, m // MCW)
    return need


def build_nc():
    """Build the Bass program for one core (SPMD; all cores run same NEFF)."""
    import concourse.bass as bass
    import concourse.tile as tile
    from concourse import bacc, mybir

    f32 = mybir.dt.float32
    bf16 = mybir.dt.bfloat16
    AF = mybir.ActivationFunctionType
    OP = mybir.AluOpType

    nc = bacc.Bacc(trn_type="TRN2", target_bir_lowering=False, debug=False)
    seqT_d = nc.dram_tensor("seqT", [128, M, BS], bf16, kind="ExternalInput")
    wT_d = nc.dram_tensor("wT", [128, M, E], bf16, kind="ExternalInput")
    cw_d = nc.dram_tensor("cw", [BS, L, K], f32, kind="ExternalInput")
    ident_d = nc.dram_tensor("ident", [128, 128], bf16, kind="ExternalInput")
    out_d = nc.dram_tensor("out", [BS, E], f32, kind="ExternalOutput")

    ENG = None  # set inside context

    with tile.TileContext(nc) as tc:
        with (
            tc.tile_pool(name="consts", bufs=1) as consts,
            tc.tile_pool(name="wtp", bufs=2) as wtp,
            tc.tile_pool(name="scrd", bufs=int(os.environ.get("KERNEL_SCRD", "4"))) as scrd,
            tc.tile_pool(name="scrg", bufs=int(os.environ.get("KERNEL_SCRG", "3"))) as scrg,
            tc.tile_pool(name="pse", bufs=3, space="PSUM") as pse,
            tc.tile_pool(name="psc", bufs=2, space="PSUM") as pscp,
        ):
            seqT = consts.tile([128, M, BS], bf16, name="seqT_sb")
            ident = consts.tile([128, 128], bf16, name="ident_sb")
            hat = consts.tile([BS, L, E], bf16, name="hat_sb")
            cw = consts.tile([BS, L, K], f32, name="cw_sb")
            wB = consts.tile([BS, L, K], bf16, name="wB_sb")
            zsum = consts.tile([BS, L], f32, name="zsum")
            zinv = consts.tile([BS, L], f32, name="zinv")
            deltaB = consts.tile([BS, L, K], f32, name="deltaB")
            capB = consts.tile([BS, E], bf16, name="capB")
            capRaw = consts.tile([BS, D, K], f32, name="capRaw")
            capAccD = (
                consts.tile([BS, D, K], f32, name="capAccD") if "D" in A_FOLD else None
            )
            capAccG = (
                consts.tile([BS, D, K], f32, name="capAccG") if "G" in A_FOLD else None
            )
            capOut = consts.tile([BS, E], f32, name="capOut")
            smalls = consts.tile([BS, 8, K], f32, name="smalls")
            nvec = smalls[:, 0, :]
            lnt = smalls[:, 1, :]
            rt = smalls[:, 2, :]
            np1 = smalls[:, 3, :]
            den = smalls[:, 4, :]
            dinv = smalls[:, 5, :]
            svec = smalls[:, 6, :]
            epsB = consts.tile([BS, 1], f32, name="epsB")
            u2s = consts.tile([BS, D], f32, name="u2s")
            nc.vector.memset(epsB[:], 1e-9)

            ENG = {"D": nc.vector, "G": nc.gpsimd}

            # ---------------- DMAs (parallel issue queues) ----------------
            # An engine-issued DMA blocks that engine for the whole transfer,
            # but transfers on different issuing engines overlap fully. cw
            # goes first on ACT (the initial softmax needs it), seqT is split
            # GPS/ACT, wT streams on SP under the einsum.
            nc.scalar.dma_start(out=cw[:], in_=cw_d[:])
            H = M // 2
            nc.gpsimd.dma_start(out=seqT[:, 0:H, :], in_=seqT_d[:, 0:H, :])

            # ---------------- helpers ----------------
            def wb_bcast(s0, nl):
                return bass.AP(
                    tensor=wB.tensor,
                    offset=wB.offset + s0 * K,
                    ap=[wB.ap[0], [K, nl], [0, D], [1, K]],
                )

            def capb_bcast(nl):
                return bass.AP(
                    tensor=capB.tensor,
                    offset=capB.offset,
                    ap=[capB.ap[0], [0, nl], [1, E]],
                )

            def zinv_bcast():
                return bass.AP(
                    tensor=zinv.tensor,
                    offset=zinv.offset,
                    ap=[zinv.ap[0], [1, L], [0, K]],
                )

            def softmax_range(s0, s1, eg=None):
                # exp straight into bf16 wB, then normalize in place
                n = s1 - s0
                zb = bass.AP(
                    tensor=zinv.tensor,
                    offset=zinv.offset + s0,
                    ap=[zinv.ap[0], [1, n], [0, K]],
                )
                nc.scalar.activation(
                    out=wB[:, s0:s1, :], in_=cw[:, s0:s1, :], func=AF.Exp
                )
                nc.vector.tensor_reduce(
                    out=zsum[:, s0:s1],
                    in_=wB[:, s0:s1, :],
                    axis=mybir.AxisListType.X,
                    op=OP.add,
                )
                nc.vector.reciprocal(out=zinv[:, s0:s1], in_=zsum[:, s0:s1])
                (eg or nc.vector).tensor_tensor(
                    out=wB[:, s0:s1, :], in0=wB[:, s0:s1, :], in1=zb, op=OP.mult
                )

            def softmax_bulk():
                softmax_range(0, L)

            def dk_col(t, k, n=D):
                # [BS, n] view of column k of a [BS, (d,k)] tile/psum region
                return bass.AP(
                    tensor=t.tensor, offset=t.offset + k, ap=[t.ap[0], [K, n]]
                )

            def squash(src, capb_src, capb_eng):
                # n[b,k] = sum_d src[b,d,k]^2 ; s = n/(1+n)/sqrt(n+1e-9)
                # src may be the PSUM accumulator directly (skips the copy on
                # the critical inter-iteration chain); capB copy runs off-chain.
                capb_eng.tensor_copy(out=capB[:], in_=capb_src)
                for k in range(K):
                    nc.vector.scalar_tensor_tensor(
                        out=u2s[:],
                        in0=dk_col(src, k),
                        scalar=1.0,
                        in1=dk_col(src, k),
                        op0=OP.mult,
                        op1=OP.mult,
                        accum_out=nvec[:, k : k + 1],
                    )
                nc.scalar.activation(out=lnt, in_=nvec, func=AF.Ln, bias=epsB[:])
                nc.scalar.activation(out=rt, in_=lnt, func=AF.Exp, scale=0.5)
                nc.vector.tensor_scalar_add(out=np1, in0=nvec, scalar1=1.0)
                nc.vector.tensor_mul(out=den, in0=np1, in1=rt)
                nc.vector.reciprocal(out=dinv, in_=den)
                nc.vector.tensor_mul(out=svec, in0=nvec, in1=dinv)

            def fold_tree_l(eng, u, nl, acc):
                """Sum u[:, 0:nl, :] over slots (nl power of 2), add into acc."""
                width = nl
                while width > 1:
                    h = width // 2
                    eng.tensor_tensor(
                        out=u[:, 0:h, :],
                        in0=u[:, 0:h, :],
                        in1=u[:, h : 2 * h, :],
                        op=OP.add,
                    )
                    width = h
                eng.tensor_tensor(
                    out=acc[:], in0=acc[:], in1=u[:, 0, :], op=OP.add
                )

            def fold_tree_d(eng, u, nl, s0):
                """delta[:, slot, k] = sum_d u[:, slot, (d,k)] -> deltaB."""
                width = D
                while width > 2:
                    h = width // 2
                    eng.tensor_tensor(
                        out=u[:, 0:nl, 0 : h * K],
                        in0=u[:, 0:nl, 0 : h * K],
                        in1=u[:, 0:nl, h * K : 2 * h * K],
                        op=OP.add,
                    )
                    width = h
                eng.tensor_tensor(
                    out=deltaB[:, s0 : s0 + nl, :],
                    in0=u[:, 0:nl, 0:K],
                    in1=u[:, 0:nl, K : 2 * K],
                    op=OP.add,
                )

            def utile(eng_key, tag):
                pool = scrd if eng_key == "D" else scrg
                return pool.tile([BS, NL, E], bf16, name=f"u{eng_key}", tag=f"u{eng_key}")

            # initial softmax (needs only cw); the ACT-issued seqT-half DMA is
            # emitted after the exp so the exp isn't queued behind it
            softmax_bulk()
            nc.scalar.dma_start(out=seqT[:, H:M, :], in_=seqT_d[:, H:M, :])
            if capAccD is not None:
                nc.vector.memset(capAccD[:], 0.0)
            if capAccG is not None:
                nc.gpsimd.memset(capAccG[:], 0.0)

            # ---------------- Phase A: einsum + hat + capacc_0 ----------------
            # chunk emission order by readiness
            order = sorted(range(NCHUNK), key=lambda c: (_chunk_ready_ci(c), c))
            pe_chunks = [c for c in order if A_FOLD[c] == "P"]
            psc0 = pscp.tile([128, 512], f32, name="psc", tag="psc")

            copy_idx = 0
            emitted = 0

            def emit_capacc0(c):
                nonlocal copy_idx
                s0, nl = _chunk_slots(c)
                me = A_MULT[c]
                u = utile(me, "a")
                ENG[me].tensor_tensor(
                    out=u[:, 0:nl, :],
                    in0=hat[:, s0 : s0 + nl, :],
                    in1=wb_bcast(s0, nl),
                    op=OP.mult,
                )
                fm = A_FOLD[c]
                if fm == "P":
                    first = c == pe_chunks[0]
                    last = c == pe_chunks[-1]
                    for j in range(nl):
                        nc.tensor.matmul(
                            psc0[:, 0:E],
                            lhsT=ident[:],
                            rhs=u[:, j, :],
                            start=(first and j == 0),
                            stop=(last and j == nl - 1),
                            skip_group_check=True,
                        )
                elif fm == "D":
                    fold_tree_l(nc.vector, u, nl, capAccD)
                else:
                    fold_tree_l(nc.gpsimd, u, nl, capAccG)

            for ci in range(M // MCW):
                m0 = ci * MCW
                wtc = wtp.tile([128, MCW, E], bf16, name="wtc", tag="wtc")
                nc.sync.dma_start(out=wtc[:], in_=wT_d[:, m0 : m0 + MCW, :])
                if ci == 0:
                    # ident is first needed by the PE folds (~12us in); keep it
                    # behind wT chunk 0 on the SP queue so einsum starts early
                    nc.sync.dma_start(out=ident[:], in_=ident_d[:])
                for par in (0, 1):
                    p0 = 64 * par
                    for g0 in range(0, MCW, PSB):
                        nb = min(PSB, MCW - g0)
                        ps = pse.tile([128, PSB, E], f32, name="pse", tag="pse")
                        for j in range(nb):
                            mo = g0 + j
                            nc.tensor.matmul(
                                ps[:, j, :],
                                lhsT=seqT[p0 : p0 + 64, m0 + mo, :],
                                rhs=wtc[p0 : p0 + 64, mo, :],
                                start=(j % 2 == 0),
                                stop=(j % 2 == 1 or j == nb - 1),
                                skip_group_check=True,
                            )
                        dst = hat[:, par * M + m0 + g0 : par * M + m0 + g0 + nb, :]
                        if copy_idx % COPY_MOD in COPY_GPS_RES:
                            nc.gpsimd.tensor_copy(out=dst, in_=ps[:, 0:nb, :])
                        else:
                            nc.scalar.copy(out=dst, in_=ps[:, 0:nb, :])
                        copy_idx += 1
                # emit routing chunks that are now fully covered
                while emitted < NCHUNK and _chunk_ready_ci(order[emitted]) <= ci:
                    emit_capacc0(order[emitted])
                    emitted += 1

            # capRaw = psc0 + tree partials
            nc.gpsimd.tensor_copy(out=capRaw[:], in_=psc0[:, 0:E])
            if "D" in A_FOLD:
                nc.vector.tensor_tensor(
                    out=capRaw[:], in0=capRaw[:], in1=capAccD[:], op=OP.add
                )
            if "G" in A_FOLD:
                nc.vector.tensor_tensor(
                    out=capRaw[:], in0=capRaw[:], in1=capAccG[:], op=OP.add
                )
            squash(
                capRaw,
                bass.AP(
                    tensor=capRaw.tensor,
                    offset=capRaw.offset,
                    ap=[capRaw.ap[0], [1, E]],
                ),
                nc.vector,
            )

            # ---------------- Phase B: iterations 1, 2 ----------------
            # Fully fused per-chunk pipeline: delta -> chunk-local cw update +
            # softmax -> capacc, whole chunk on one engine (exp on ACT, folds
            # of capacc on PE), so the PE ident-matmul stream and the ACT exps
            # hide under the DVE/GPSIMD streams with no bulk barriers.
            def svec_bcast(nl):
                return bass.AP(
                    tensor=smalls.tensor,
                    offset=smalls.offset + 6 * K,
                    ap=[smalls.ap[0], [0, nl], [1, K]],
                )

            def fused_chunk(c, psc, first, last):
                s0, nl = _chunk_slots(c)
                me = B_ENG[c]
                EG = ENG[me]
                u = utile(me, "b")
                EG.tensor_tensor(
                    out=u[:, 0:nl, :],
                    in0=hat[:, s0 : s0 + nl, :],
                    in1=capb_bcast(nl),
                    op=OP.mult,
                )
                fold_tree_d(EG, u, nl, s0)
                # cw_c += svec * deltaB_c (product built in deltaB in place)
                EG.tensor_tensor(
                    out=deltaB[:, s0 : s0 + nl, :],
                    in0=deltaB[:, s0 : s0 + nl, :],
                    in1=svec_bcast(nl),
                    op=OP.mult,
                )
                EG.tensor_tensor(
                    out=cw[:, s0 : s0 + nl, :],
                    in0=cw[:, s0 : s0 + nl, :],
                    in1=deltaB[:, s0 : s0 + nl, :],
                    op=OP.add,
                )
                # chunk softmax. exp on ACT; for GPSIMD chunks the k-sum and
                # normalize stay on GPSIMD (adds + divide) so the chain never
                # waits in the saturated DVE queue.
                if me == "G":
                    nc.scalar.activation(
                        out=wB[:, s0 : s0 + nl, :],
                        in_=cw[:, s0 : s0 + nl, :],
                        func=AF.Exp,
                    )
                    zs = zsum[:, s0 : s0 + nl]

                    def k_col(k):
                        return bass.AP(
                            tensor=wB.tensor,
                            offset=wB.offset + s0 * K + k,
                            ap=[wB.ap[0], [K, nl]],
                        )

                    EG.tensor_tensor(out=zs, in0=k_col(0), in1=k_col(1), op=OP.add)
                    EG.tensor_tensor(out=zs, in0=zs, in1=k_col(2), op=OP.add)
                    EG.tensor_tensor(out=zs, in0=zs, in1=k_col(3), op=OP.add)
                    zb = bass.AP(
                        tensor=zsum.tensor,
                        offset=zsum.offset + s0,
                        ap=[zsum.ap[0], [1, nl], [0, K]],
                    )
                    EG.tensor_tensor(
                        out=wB[:, s0 : s0 + nl, :],
                        in0=wB[:, s0 : s0 + nl, :],
                        in1=zb,
                        op=OP.divide,
                    )
                else:
                    softmax_range(s0, s0 + nl, eg=EG)
                # capacc
                u3 = utile(me, "b")
                EG.tensor_tensor(
                    out=u3[:, 0:nl, :],
                    in0=hat[:, s0 : s0 + nl, :],
                    in1=wb_bcast(s0, nl),
                    op=OP.mult,
                )
                for j in range(nl):
                    nc.tensor.matmul(
                        psc[:, 0:E],
                        lhsT=ident[:],
                        rhs=u3[:, j, :],
                        start=(first and j == 0),
                        stop=(last and j == nl - 1),
                        skip_group_check=True,
                    )

            for it in (1, 2):
                psc = pscp.tile([128, 512], f32, name="psc", tag="psc")
                for c in range(NCHUNK):
                    fused_chunk(c, psc, first=(c == 0), last=(c == NCHUNK - 1))
                squash(psc, psc[:, 0:E], nc.gpsimd)
                if it == 2:
                    # final: out[b, (k,d)] = s[b,k] * psc[b, (d,k)]
                    for k in range(K):
                        nc.vector.tensor_scalar_mul(
                            out=capOut[:, k * D : (k + 1) * D],
                            in0=dk_col(psc, k),
                            scalar1=svec[:, k : k + 1],
                        )
            nc.sync.dma_start(out=out_d[:], in_=capOut[:])

    nc.finalize()
    return nc


_NC_CACHE = None


def _get_nc():
    global _NC_CACHE
    if _NC_CACHE is None:
        _NC_CACHE = build_nc()
    return _NC_CACHE


def prep_inputs(seq_out, weights, capsule_weight):
    """Host-side layout prep -> list of per-core input maps."""
    import ml_dtypes

    bf16 = ml_dtypes.bfloat16
    seq = np.ascontiguousarray(np.asarray(seq_out, dtype=np.float32))
    W = np.ascontiguousarray(np.asarray(weights, dtype=np.float32))[0]  # [L,E,D]
    cwf = np.ascontiguousarray(np.asarray(capsule_weight, dtype=np.float32))

    # seqT[p=(64*par+d'), m, b] = seq[b, 2m+par, d']
    seqT = np.ascontiguousarray(
        seq.reshape(B, M, 2, D).transpose(2, 3, 1, 0).reshape(128, M, B).astype(bf16)
    )
    # wT[p, m, (d*K+k)] = W[2m+par, k*D+d, d']
    wTf = W.reshape(M, 2, K, D, D).transpose(1, 4, 0, 3, 2)  # [par, d', m, d, k]
    wT = np.ascontiguousarray(wTf.reshape(128, M, E).astype(bf16))
    # cwA[b, slot=(par*M+m), k] = cw[b, k, 2m+par]
    cwA = np.ascontiguousarray(
        cwf.reshape(B, K, M, 2).transpose(0, 3, 2, 1).reshape(B, L, K)
    )
    ident = np.eye(128, dtype=bf16)

    in_maps = []
    for c in range(NCORES):
        in_maps.append(
            {
                "seqT": np.ascontiguousarray(seqT[:, :, c * BS : (c + 1) * BS]),
                "wT": wT,
                "cw": np.ascontiguousarray(cwA[c * BS : (c + 1) * BS]),
                "ident": ident,
            }
        )
    return in_maps


def gather_out(results):
    """Per-core 'out' [BS, E=(k*D+d)] -> full [B, K, D]."""
    return np.concatenate(
        [np.asarray(r["out"]).reshape(BS, K, D) for r in results], axis=0
    ).astype(np.float32)


def kernel(seq_out, mask, weights, capsule_weight):
    from concourse.bass_utils import run_bass_kernel_spmd

    nc = _get_nc()
    in_maps = prep_inputs(seq_out, weights, capsule_weight)
    res = run_bass_kernel_spmd(nc, in_maps, core_ids=list(range(NCORES)))
    return gather_out(res.results)


if __name__ == "__main__":
    rng = np.random.default_rng(0)
    seq_out = rng.standard_normal((B, L, D), dtype=np.float32)
    mask = np.ones((B, L), dtype=np.float32)
    weights = (0.02 * rng.standard_normal((1, L, E, D))).astype(np.float32)
    capsule_weight = rng.standard_normal((B, K, L)).astype(np.float32)
    out = kernel(seq_out, mask, weights, capsule_weight)
    print("out", out.shape, out.dtype, float(np.abs(out).max()))


# revision 42
# speedup vs baseline: 1.2796x; 1.0017x over previous
"""Trainium2 Bass kernel for the CapsuleNetwork routing problem (v2).

Problem (per reference):
  B, L, D, K = 1024, 200, 64, 4 ; E = K*D = 256
  hat[b,l,e] = sum_d seq[b,l,d] * W[l,e,d]          (einsum, PE)
  3 rounds of dynamic routing over interests K (softmax over K per (b,l)),
  cap = squash(w @ hat), cw += hat . cap
  output cap -> [B, K, D]

Sharding: pure data-parallel over batch across 8 NeuronCores (BS=128 rows
each); weights replicated. Host-side layout prep gives the device clean
burst DMAs and parity-major (par, m) slot order, l = 2m + par.

v2 architecture (vs the v1 3x-einsum-recompute kernel):
  * All inputs bf16 (halves DMA bytes); DMAs issued from SP + ACT + GPSIMD
    queues concurrently (transfers overlap across issuing engines).
  * hat computed ONCE (bf16, SBUF-resident, 100KiB/partition) -- einsum and
    the PSUM->SBUF copies happen once instead of 3x.
  * The over-l capacc reduction runs on the otherwise-idle PE as
    identity-matmul PSUM accumulation (fp32, better precision than bf16
    trees), freeing the DVE.
  * Remaining elementwise work (routing mults + delta d-folds) is split
    DVE/GPSIMD by per-chunk schedule tables (GPSIMD ~1.5x DVE cost/elem in
    the cost model but otherwise idle).
  * cw update + softmax run bulk per iteration (few big ops), not per chunk.

Restructured routing algebra (validated in v1 vs reference to ~3e-7):
  cw layout [B, slot, K];  w = exp(cw) / sum_k exp(cw)
  capRaw[b,(d,k)] = sum_slot w[b,slot,k] hat[b,slot,(d,k)]
  n = |capRaw|^2 ; s = n/(1+n)/sqrt(n+1e-9)
  cw += s[b,k] * (hat . capRaw)   (squash scale folded into the cw update)
  final out[b,(k,d)] = s * capRaw
"""

import os
import sys

import numpy as np

for _p in ("/opt/trn_rl_repo", "/root/.axon_site/_ro/trn_rl_repo"):
    if os.path.isdir(_p) and _p not in sys.path:
        sys.path.insert(0, _p)

B, L, D, K = 1024, 200, 64, 4
E = K * D
NCORES = 8
BS = B // NCORES  # 128 batch rows per core
M = L // 2        # l = 2m + par ; slot = par*M + m

MCW = 10   # m's per wT DMA chunk (10 chunks)
PSB = 4    # m's per einsum PSUM tile (single parity)
NL = 16    # slots per routing chunk -> 13 chunks (12x16 + 8)

NCHUNK = (L + NL - 1) // NL


def _sched(env, default):
    s = os.environ.get(env, default)
    out = s.split(",")
    assert len(out) == NCHUNK, f"{env}: need {NCHUNK} entries, got {len(out)}"
    return out


# Engine schedules: D=DVE, G=GPSIMD, P=PE(identity-matmul fold), A=ACT.
A_MULT = _sched("KERNEL_A_MULT", "D,D,D,D,D,D,D,D,D,D,D,G,G")
A_FOLD = _sched("KERNEL_A_FOLD", "P,D,P,G,D,P,P,D,P,D,P,D,P")
# phase-B whole-chunk engine assignment (fused per-chunk pipeline)
B_ENG = _sched("KERNEL_B_ENG", "D,G,D,G,D,G,D,G,D,G,D,D,D")
# einsum PSUM->SBUF hat copy engine: copies with idx%COPY_MOD in RES -> GPS
COPY_MOD = int(os.environ.get("KERNEL_COPY_MOD", "2"))
COPY_GPS_RES = tuple(
    int(x) for x in os.environ.get("KERNEL_COPY_GPS_RES", "1").split(",") if x != ""
)


def _chunk_slots(c):
    s0 = NL * c
    return s0, min(NL, L - s0)


def _chunk_ready_ci(c):
    """First wT-chunk index ci after which einsum has produced all hat slots
    of routing chunk c (einsum emits both parities per ci block)."""
    s0, nl = _chunk_slots(c)
    need = 0
    for s in range(s0, s0 + nl):
        m = s % M
        need = max(need, m // MCW)
    return need


def build_nc():
    """Build the Bass program for one core (SPMD; all cores run same NEFF)."""
    import concourse.bass as bass
    import concourse.tile as tile
    from concourse import bacc, mybir

    f32 = mybir.dt.float32
    bf16 = mybir.dt.bfloat16
    AF = mybir.ActivationFunctionType
    OP = mybir.AluOpType

    nc = bacc.Bacc(trn_type="TRN2", target_bir_lowering=False, debug=False)
    seqT_d = nc.dram_tensor("seqT", [128, M, BS], bf16, kind="ExternalInput")
    wT_d = nc.dram_tensor("wT", [128, M, E], bf16, kind="ExternalInput")
    cw_d = nc.dram_tensor("cw", [BS, L, K], f32, kind="ExternalInput")
    ident_d = nc.dram_tensor("ident", [128, 128], bf16, kind="ExternalInput")
    out_d = nc.dram_tensor("out", [BS, E], f32, kind="ExternalOutput")

    ENG = None  # set inside context

    with tile.TileContext(nc) as tc:
        with (
            tc.tile_pool(name="consts", bufs=1) as consts,
            tc.tile_pool(name="wtp", bufs=2) as wtp,
            tc.tile_pool(name="scrd", bufs=int(os.environ.get("KERNEL_SCRD", "4"))) as scrd,
            tc.tile_pool(name="scrg", bufs=int(os.environ.get("KERNEL_SCRG", "3"))) as scrg,
            tc.tile_pool(name="pse", bufs=3, space="PSUM") as pse,
            tc.tile_pool(name="psc", bufs=2, space="PSUM") as pscp,
        ):
            seqT = consts.tile([128, M, BS], bf16, name="seqT_sb")
            ident = consts.tile([128, 128], bf16, name="ident_sb")
            hat = consts.tile([BS, L, E], bf16, name="hat_sb")
            cw = consts.tile([BS, L, K], f32, name="cw_sb")
            wB = consts.tile([BS, L, K], bf16, name="wB_sb")
            zsum = consts.tile([BS, L], f32, name="zsum")
            zinv = consts.tile([BS, L], f32, name="zinv")
            deltaB = consts.tile([BS, L, K], f32, name="deltaB")
            capB = consts.tile([BS, E], bf16, name="capB")
            capRaw = consts.tile([BS, D, K], f32, name="capRaw")
            capAccD = (
                consts.tile([BS, D, K], f32, name="capAccD") if "D" in A_FOLD else None
            )
            capAccG = (
                consts.tile([BS, D, K], f32, name="capAccG") if "G" in A_FOLD else None
            )
            capOut = consts.tile([BS, E], f32, name="capOut")
            smalls = consts.tile([BS, 8, K], f32, name="smalls")
            nvec = smalls[:, 0, :]
            lnt = smalls[:, 1, :]
            rt = smalls[:, 2, :]
            np1 = smalls[:, 3, :]
            den = smalls[:, 4, :]
            dinv = smalls[:, 5, :]
            svec = smalls[:, 6, :]
            epsB = consts.tile([BS, 1], f32, name="epsB")
            u2s = consts.tile([BS, D], f32, name="u2s")
            nc.vector.memset(epsB[:], 1e-9)

            ENG = {"D": nc.vector, "G": nc.gpsimd}

            # ---------------- DMAs (parallel issue queues) ----------------
            # An engine-issued DMA blocks that engine for the whole transfer,
            # but transfers on different issuing engines overlap fully. cw
            # goes first on ACT (the initial softmax needs it), seqT is split
            # GPS/ACT, wT streams on SP under the einsum.
            nc.scalar.dma_start(out=cw[:], in_=cw_d[:])
            H = M // 2
            nc.gpsimd.dma_start(out=seqT[:, 0:H, :], in_=seqT_d[:, 0:H, :])

            # ---------------- helpers ----------------
            def wb_bcast(s0, nl):
                return bass.AP(
                    tensor=wB.tensor,
                    offset=wB.offset + s0 * K,
                    ap=[wB.ap[0], [K, nl], [0, D], [1, K]],
                )

            def capb_bcast(nl):
                return bass.AP(
                    tensor=capB.tensor,
                    offset=capB.offset,
                    ap=[capB.ap[0], [0, nl], [1, E]],
                )

            def zinv_bcast():
                return bass.AP(
                    tensor=zinv.tensor,
                    offset=zinv.offset,
                    ap=[zinv.ap[0], [1, L], [0, K]],
                )

            def softmax_range(s0, s1, eg=None):
                # exp straight into bf16 wB, then normalize in place
                n = s1 - s0
                zb = bass.AP(
                    tensor=zinv.tensor,
                    offset=zinv.offset + s0,
                    ap=[zinv.ap[0], [1, n], [0, K]],
                )
                nc.scalar.activation(
                    out=wB[:, s0:s1, :], in_=cw[:, s0:s1, :], func=AF.Exp
                )
                nc.vector.tensor_reduce(
                    out=zsum[:, s0:s1],
                    in_=wB[:, s0:s1, :],
                    axis=mybir.AxisListType.X,
                    op=OP.add,
                )
                nc.vector.reciprocal(out=zinv[:, s0:s1], in_=zsum[:, s0:s1])
                (eg or nc.vector).tensor_tensor(
                    out=wB[:, s0:s1, :], in0=wB[:, s0:s1, :], in1=zb, op=OP.mult
                )

            def softmax_bulk():
                softmax_range(0, L)

            def dk_col(t, k, n=D):
                # [BS, n] view of column k of a [BS, (d,k)] tile/psum region
                return bass.AP(
                    tensor=t.tensor, offset=t.offset + k, ap=[t.ap[0], [K, n]]
                )

            def squash(src, capb_src, capb_eng):
                # n[b,k] = sum_d src[b,d,k]^2 ; s = n/(1+n)/sqrt(n+1e-9)
                # src may be the PSUM accumulator directly (skips the copy on
                # the critical inter-iteration chain); capB copy runs off-chain.
                capb_eng.tensor_copy(out=capB[:], in_=capb_src)
                for k in range(K):
                    nc.vector.scalar_tensor_tensor(
                        out=u2s[:],
                        in0=dk_col(src, k),
                        scalar=1.0,
                        in1=dk_col(src, k),
                        op0=OP.mult,
                        op1=OP.mult,
                        accum_out=nvec[:, k : k + 1],
                    )
                nc.scalar.activation(out=lnt, in_=nvec, func=AF.Ln, bias=epsB[:])
                nc.scalar.activation(out=rt, in_=lnt, func=AF.Exp, scale=0.5)
                nc.vector.tensor_scalar_add(out=np1, in0=nvec, scalar1=1.0)
                nc.vector.tensor_mul(out=den, in0=np1, in1=rt)
                nc.vector.reciprocal(out=dinv, in_=den)
                nc.vector.tensor_mul(out=svec, in0=nvec, in1=dinv)

            def fold_tree_l(eng, u, nl, acc):
                """Sum u[:, 0:nl, :] over slots (nl power of 2), add into acc."""
                width = nl
                while width > 1:
                    h = width // 2
                    eng.tensor_tensor(
                        out=u[:, 0:h, :],
                        in0=u[:, 0:h, :],
                        in1=u[:, h : 2 * h, :],
                        op=OP.add,
                    )
                    width = h
                eng.tensor_tensor(
                    out=acc[:], in0=acc[:], in1=u[:, 0, :], op=OP.add
                )

            def fold_tree_d(eng, u, nl, s0):
                """delta[:, slot, k] = sum_d u[:, slot, (d,k)] -> deltaB."""
                width = D
                while width > 2:
                    h = width // 2
                    eng.tensor_tensor(
                        out=u[:, 0:nl, 0 : h * K],
                        in0=u[:, 0:nl, 0 : h * K],
                        in1=u[:, 0:nl, h * K : 2 * h * K],
                        op=OP.add,
                    )
                    width = h
                eng.tensor_tensor(
                    out=deltaB[:, s0 : s0 + nl, :],
                    in0=u[:, 0:nl, 0:K],
                    in1=u[:, 0:nl, K : 2 * K],
                    op=OP.add,
                )

            def utile(eng_key, tag):
                pool = scrd if eng_key == "D" else scrg
                return pool.tile([BS, NL, E], bf16, name=f"u{eng_key}", tag=f"u{eng_key}")

            # initial softmax (needs only cw); the ACT-issued seqT-half DMA is
            # emitted after the exp so the exp isn't queued behind it
            softmax_bulk()
            nc.scalar.dma_start(out=seqT[:, H:M, :], in_=seqT_d[:, H:M, :])
            if capAccD is not None:
                nc.vector.memset(capAccD[:], 0.0)
            if capAccG is not None:
                nc.gpsimd.memset(capAccG[:], 0.0)

            # ---------------- Phase A: einsum + hat + capacc_0 ----------------
            # chunk emission order by readiness
            order = sorted(range(NCHUNK), key=lambda c: (_chunk_ready_ci(c), c))
            pe_chunks = [c for c in order if A_FOLD[c] == "P"]
            psc0 = pscp.tile([128, 512], f32, name="psc", tag="psc")

            copy_idx = 0
            emitted = 0

            def emit_capacc0(c):
                nonlocal copy_idx
                s0, nl = _chunk_slots(c)
                me = A_MULT[c]
                u = utile(me, "a")
                ENG[me].tensor_tensor(
                    out=u[:, 0:nl, :],
                    in0=hat[:, s0 : s0 + nl, :],
                    in1=wb_bcast(s0, nl),
                    op=OP.mult,
                )
                fm = A_FOLD[c]
                if fm == "P":
                    first = c == pe_chunks[0]
                    last = c == pe_chunks[-1]
                    for j in range(nl):
                        nc.tensor.matmul(
                            psc0[:, 0:E],
                            lhsT=ident[:],
                            rhs=u[:, j, :],
                            start=(first and j == 0),
                            stop=(last and j == nl - 1),
                            skip_group_check=True,
                        )
                elif fm == "D":
                    fold_tree_l(nc.vector, u, nl, capAccD)
                else:
                    fold_tree_l(nc.gpsimd, u, nl, capAccG)

            for ci in range(M // MCW):
                m0 = ci * MCW
                wtc = wtp.tile([128, MCW, E], bf16, name="wtc", tag="wtc")
                nc.sync.dma_start(out=wtc[:], in_=wT_d[:, m0 : m0 + MCW, :])
                if ci == 0:
                    # ident is first needed by the PE folds (~12us in); keep it
                    # behind wT chunk 0 on the SP queue so einsum starts early
                    nc.sync.dma_start(out=ident[:], in_=ident_d[:])
                for par in (0, 1):
                    p0 = 64 * par
                    for g0 in range(0, MCW, PSB):
                        nb = min(PSB, MCW - g0)
                        ps = pse.tile([128, PSB, E], f32, name="pse", tag="pse")
                        for j in range(nb):
                            mo = g0 + j
                            nc.tensor.matmul(
                                ps[:, j, :],
                                lhsT=seqT[p0 : p0 + 64, m0 + mo, :],
                                rhs=wtc[p0 : p0 + 64, mo, :],
                                start=(j % 2 == 0),
                                stop=(j % 2 == 1 or j == nb - 1),
                                skip_group_check=True,
                            )
                        dst = hat[:, par * M + m0 + g0 : par * M + m0 + g0 + nb, :]
                        if copy_idx < 12 and copy_idx % 3 == 2:
                            # early copies: DVE is idle during the ramp; a third
                            # copy engine un-gates the first capacc_0 chunks
                            nc.vector.tensor_copy(out=dst, in_=ps[:, 0:nb, :])
                        elif copy_idx % COPY_MOD in COPY_GPS_RES:
                            nc.gpsimd.tensor_copy(out=dst, in_=ps[:, 0:nb, :])
                        else:
                            nc.scalar.copy(out=dst, in_=ps[:, 0:nb, :])
                        copy_idx += 1
                # emit routing chunks that are now fully covered
                while emitted < NCHUNK and _chunk_ready_ci(order[emitted]) <= ci:
                    emit_capacc0(order[emitted])
                    emitted += 1

            # capRaw = psc0 + tree partials
            nc.gpsimd.tensor_copy(out=capRaw[:], in_=psc0[:, 0:E])
            if "D" in A_FOLD:
                nc.vector.tensor_tensor(
                    out=capRaw[:], in0=capRaw[:], in1=capAccD[:], op=OP.add
                )
            if "G" in A_FOLD:
                nc.vector.tensor_tensor(
                    out=capRaw[:], in0=capRaw[:], in1=capAccG[:], op=OP.add
                )
            squash(
                capRaw,
                bass.AP(
                    tensor=capRaw.tensor,
                    offset=capRaw.offset,
                    ap=[capRaw.ap[0], [1, E]],
                ),
                nc.gpsimd,
            )

            # ---------------- Phase B: iterations 1, 2 ----------------
            # Fully fused per-chunk pipeline: delta -> chunk-local cw update +
            # softmax -> capacc, whole chunk on one engine (exp on ACT, folds
            # of capacc on PE), so the PE ident-matmul stream and the ACT exps
            # hide under the DVE/GPSIMD streams with no bulk barriers.
            def svec_bcast(nl):
                return bass.AP(
                    tensor=smalls.tensor,
                    offset=smalls.offset + 6 * K,
                    ap=[smalls.ap[0], [0, nl], [1, K]],
                )

            def fused_chunk(c, psc, first, last):
                s0, nl = _chunk_slots(c)
                me = B_ENG[c]
                EG = ENG[me]
                u = utile(me, "b")
                EG.tensor_tensor(
                    out=u[:, 0:nl, :],
                    in0=hat[:, s0 : s0 + nl, :],
                    in1=capb_bcast(nl),
                    op=OP.mult,
                )
                fold_tree_d(EG, u, nl, s0)
                # cw_c += svec * deltaB_c (product built in deltaB in place)
                EG.tensor_tensor(
                    out=deltaB[:, s0 : s0 + nl, :],
                    in0=deltaB[:, s0 : s0 + nl, :],
                    in1=svec_bcast(nl),
                    op=OP.mult,
                )
                EG.tensor_tensor(
                    out=cw[:, s0 : s0 + nl, :],
                    in0=cw[:, s0 : s0 + nl, :],
                    in1=deltaB[:, s0 : s0 + nl, :],
                    op=OP.add,
                )
                # chunk softmax. exp on ACT; for GPSIMD chunks the k-sum and
                # normalize stay on GPSIMD (adds + divide) so the chain never
                # waits in the saturated DVE queue.
                if me == "G":
                    nc.scalar.activation(
                        out=wB[:, s0 : s0 + nl, :],
                        in_=cw[:, s0 : s0 + nl, :],
                        func=AF.Exp,
                    )
                    zs = zsum[:, s0 : s0 + nl]

                    def k_col(k):
                        return bass.AP(
                            tensor=wB.tensor,
                            offset=wB.offset + s0 * K + k,
                            ap=[wB.ap[0], [K, nl]],
                        )

                    EG.tensor_tensor(out=zs, in0=k_col(0), in1=k_col(1), op=OP.add)
                    EG.tensor_tensor(out=zs, in0=zs, in1=k_col(2), op=OP.add)
                    EG.tensor_tensor(out=zs, in0=zs, in1=k_col(3), op=OP.add)
                    zb = bass.AP(
                        tensor=zsum.tensor,
                        offset=zsum.offset + s0,
                        ap=[zsum.ap[0], [1, nl], [0, K]],
                    )
                    EG.tensor_tensor(
                        out=wB[:, s0 : s0 + nl, :],
                        in0=wB[:, s0 : s0 + nl, :],
                        in1=zb,
                        op=OP.divide,
                    )
                else:
                    softmax_range(s0, s0 + nl, eg=EG)
                # capacc
                u3 = utile(me, "b")
                EG.tensor_tensor(
                    out=u3[:, 0:nl, :],
                    in0=hat[:, s0 : s0 + nl, :],
                    in1=wb_bcast(s0, nl),
                    op=OP.mult,
                )
                for j in range(nl):
                    nc.tensor.matmul(
                        psc[:, 0:E],
                        lhsT=ident[:],
                        rhs=u3[:, j, :],
                        start=(first and j == 0),
                        stop=(last and j == nl - 1),
                        skip_group_check=True,
                    )

            for it in (1, 2):
                psc = pscp.tile([128, 512], f32, name="psc", tag="psc")
                for c in range(NCHUNK):
                    fused_chunk(c, psc, first=(c == 0), last=(c == NCHUNK - 1))
                squash(psc, psc[:, 0:E], nc.gpsimd)
                if it == 2:
                    # final: out[b, (k,d)] = s[b,k] * psc[b, (d,k)]
                    for k in range(K):
                        nc.vector.tensor_scalar_mul(
                            out=capOut[:, k * D : (k + 1) * D],
                            in0=dk_col(psc, k),
                            scalar1=svec[:, k : k + 1],
                        )
            nc.sync.dma_start(out=out_d[:], in_=capOut[:])

    nc.finalize()
    return nc


_NC_CACHE = None


def _get_nc():
    global _NC_CACHE
    if _NC_CACHE is None:
        _NC_CACHE = build_nc()
    return _NC_CACHE


def prep_inputs(seq_out, weights, capsule_weight):
    """Host-side layout prep -> list of per-core input maps."""
    import ml_dtypes

    bf16 = ml_dtypes.bfloat16
    seq = np.ascontiguousarray(np.asarray(seq_out, dtype=np.float32))
    W = np.ascontiguousarray(np.asarray(weights, dtype=np.float32))[0]  # [L,E,D]
    cwf = np.ascontiguousarray(np.asarray(capsule_weight, dtype=np.float32))

    # seqT[p=(64*par+d'), m, b] = seq[b, 2m+par, d']
    seqT = np.ascontiguousarray(
        seq.reshape(B, M, 2, D).transpose(2, 3, 1, 0).reshape(128, M, B).astype(bf16)
    )
    # wT[p, m, (d*K+k)] = W[2m+par, k*D+d, d']
    wTf = W.reshape(M, 2, K, D, D).transpose(1, 4, 0, 3, 2)  # [par, d', m, d, k]
    wT = np.ascontiguousarray(wTf.reshape(128, M, E).astype(bf16))
    # cwA[b, slot=(par*M+m), k] = cw[b, k, 2m+par]
    cwA = np.ascontiguousarray(
        cwf.reshape(B, K, M, 2).transpose(0, 3, 2, 1).reshape(B, L, K)
    )
    ident = np.eye(128, dtype=bf16)

    in_maps = []
    for c in range(NCORES):
        in_maps.append(
            {
                "seqT": np.ascontiguousarray(seqT[:, :, c * BS : (c + 1) * BS]),
                "wT": wT,
                "cw": np.ascontiguousarray(cwA[c * BS : (c + 1) * BS]),
                "ident": ident,
            }
        )
    return in_maps


def gather_out(results):
    """Per-core 'out' [BS, E=(k*D+d)] -> full [B, K, D]."""
    return np.concatenate(
        [np.asarray(r["out"]).reshape(BS, K, D) for r in results], axis=0
    ).astype(np.float32)


def kernel(seq_out, mask, weights, capsule_weight):
    from concourse.bass_utils import run_bass_kernel_spmd

    nc = _get_nc()
    in_maps = prep_inputs(seq_out, weights, capsule_weight)
    res = run_bass_kernel_spmd(nc, in_maps, core_ids=list(range(NCORES)))
    return gather_out(res.results)


if __name__ == "__main__":
    rng = np.random.default_rng(0)
    seq_out = rng.standard_normal((B, L, D), dtype=np.float32)
    mask = np.ones((B, L), dtype=np.float32)
    weights = (0.02 * rng.standard_normal((1, L, E, D))).astype(np.float32)
    capsule_weight = rng.standard_normal((B, K, L)).astype(np.float32)
    out = kernel(seq_out, mask, weights, capsule_weight)
    print("out", out.shape, out.dtype, float(np.abs(out).max()))
